# revision 7
# baseline (speedup 1.0000x reference)
"""ConvTranBackbone TRN2 Bass kernel — data-parallel over batch on 8 NeuronCores.

Per core: 2 batches of conv1(1->512,k7) + BN + gelu -> conv2(512x16 contraction)
+ BN + gelu -> tAPE -> rel-bias MHA (softmax then +bias) -> LN -> FF -> LN -> [D,L].
Matmuls run in float32r (full PE rate, ~1e-4 rounding); reductions in fp32 PSUM.
"""
import math
import numpy as np
from contextlib import ExitStack

import concourse.bass as bass
import concourse.tile as tile
import concourse.mybir as mybir
from concourse import bacc
from concourse.bass_utils import run_bass_kernel_spmd

F32 = mybir.dt.float32
F32R = mybir.dt.float32r
AF = mybir.ActivationFunctionType
OP = mybir.AluOpType

B, C_IN, L, D, H, DFF = 16, 16, 1024, 128, 8, 512
D4 = 512
EPS = 1e-5
N_CORES = 8
BPC = B // N_CORES          # batches per core
DH = D // H                 # 16
SCALE = D ** -0.5

_CACHE = {}


def _layer_norm_tiles(nc, sb, z_tiles, g_bc, b_bc, out_tiles, tag):
    """LN over free dim (128) for 8 [128,128] f32 tiles."""
    mu = sb.tile([128, 8], F32, name=f"ln_mu_{tag}", tag="ln_mu")
    var = sb.tile([128, 8], F32, name=f"ln_var_{tag}", tag="ln_var")
    xc = sb.tile([128, 8, 128], F32, name=f"ln_xc_{tag}", tag="ln_xc")
    sq = sb.tile([128, 128], F32, name=f"ln_sq_{tag}", tag="ln_sq", bufs=2)
    for lt in range(8):
        z = z_tiles(lt)
        nc.vector.tensor_reduce(mu[:, lt:lt + 1], z, axis=mybir.AxisListType.X,
                                op=OP.add)
    nc.vector.tensor_scalar_mul(mu[:], mu[:], 1.0 / 128)
    for lt in range(8):
        z = z_tiles(lt)
        nc.vector.tensor_scalar(out=xc[:, lt, :], in0=z,
                                scalar1=mu[:, lt:lt + 1], scalar2=None,
                                op0=OP.subtract)
        nc.vector.tensor_tensor(sq[:], xc[:, lt, :], xc[:, lt, :], op=OP.mult)
        nc.vector.tensor_reduce(var[:, lt:lt + 1], sq[:],
                                axis=mybir.AxisListType.X, op=OP.add)
    nc.vector.tensor_scalar(out=var[:], in0=var[:], scalar1=1.0 / 128,
                            scalar2=EPS, op0=OP.mult, op1=OP.add)
    std = sb.tile([128, 8], F32, name=f"ln_std_{tag}", tag="ln_std")
    nc.scalar.activation(std[:], var[:], AF.Sqrt)
    istd = sb.tile([128, 8], F32, name=f"ln_istd_{tag}", tag="ln_istd")
    nc.vector.reciprocal(istd[:], std[:])
    for lt in range(8):
        o = out_tiles(lt)
        nc.vector.tensor_scalar_mul(xc[:, lt, :], xc[:, lt, :],
                                    istd[:, lt:lt + 1])
        nc.vector.tensor_tensor(xc[:, lt, :], xc[:, lt, :], g_bc[:], op=OP.mult)
        nc.vector.tensor_tensor(o, xc[:, lt, :], b_bc[:], op=OP.add)


def _build():
    import os
    PH = os.environ.get("KPHASES", "ABCD")
    DL = int(os.environ.get("KDLVL", "4"))
    nc = bacc.Bacc("TRN2", target_bir_lowering=False, debug=False,
                   num_devices=N_CORES)

    def din(name, shape, dt=F32R):
        return nc.dram_tensor(name, shape, dt, kind="ExternalInput").ap()

    xpad = din("xpad", [BPC, C_IN, L + 6])
    w1r = din("w1r", [7, D4])
    s1t = din("s1t", [128, 4], F32)
    b1e = din("b1e", [128, 4], F32)
    w2l = din("w2l", [128, 64, 128])
    s2t = din("s2t", [128, 1], F32)
    b2e = din("b2e", [128, 1], F32)
    peT = din("peT", [128, L])
    wqT = din("wqT", [128, 128])
    wkT = din("wkT", [128, 128])
    wvT = din("wvT", [128, 128])
    ttr = din("ttr", [128, H, 2047])
    w1ft = din("w1ft", [128, 4, 128])
    b1ft = din("b1ft", [128, 4], F32)
    w2ft = din("w2ft", [128, 4, 128])
    b2bc = din("b2bc", [128, 128], F32)
    lag = din("lag", [128, 128], F32)
    lab = din("lab", [128, 128], F32)
    l1g = din("l1g", [128, 128], F32)
    l1b = din("l1b", [128, 128], F32)
    l2g = din("l2g", [128, 128], F32)
    l2b = din("l2b", [128, 128], F32)
    idr = din("idr", [128, 128])
    idf = din("idf", [128, 128], F32)
    yout = nc.dram_tensor("yout", [BPC, D, L], F32, kind="ExternalOutput").ap()

    with tile.TileContext(nc) as tc, ExitStack() as ctx:
        res = ctx.enter_context(tc.tile_pool(name="res", bufs=1))
        sb = ctx.enter_context(tc.tile_pool(name="sb", bufs=1))

        # ---------- residents ----------
        W1 = res.tile([7, D4], F32R)
        nc.sync.dma_start(W1[:], w1r[:])
        W2L = res.tile([128, 64, 128], F32R)
        nc.sync.dma_start(W2L[:], w2l[:])
        S1 = res.tile([128, 4], F32); nc.sync.dma_start(S1[:], s1t[:])
        B1 = res.tile([128, 4], F32); nc.sync.dma_start(B1[:], b1e[:])
        S2 = res.tile([128, 1], F32); nc.sync.dma_start(S2[:], s2t[:])
        B2 = res.tile([128, 1], F32); nc.sync.dma_start(B2[:], b2e[:])
        PET = res.tile([128, L], F32R); nc.sync.dma_start(PET[:], peT[:])
        WQT = res.tile([128, 128], F32R); nc.sync.dma_start(WQT[:], wqT[:])
        WKT = res.tile([128, 128], F32R); nc.sync.dma_start(WKT[:], wkT[:])
        WVT = res.tile([128, 128], F32R); nc.sync.dma_start(WVT[:], wvT[:])
        W1F = res.tile([128, 4, 128], F32R); nc.sync.dma_start(W1F[:], w1ft[:])
        B1F = res.tile([128, 4], F32); nc.sync.dma_start(B1F[:], b1ft[:])
        W2F = res.tile([128, 4, 128], F32R); nc.sync.dma_start(W2F[:], w2ft[:])
        B2BC = res.tile([128, 128], F32); nc.sync.dma_start(B2BC[:], b2bc[:])
        LAG = res.tile([128, 128], F32); nc.sync.dma_start(LAG[:], lag[:])
        LAB = res.tile([128, 128], F32); nc.sync.dma_start(LAB[:], lab[:])
        L1G = res.tile([128, 128], F32); nc.sync.dma_start(L1G[:], l1g[:])
        L1B = res.tile([128, 128], F32); nc.sync.dma_start(L1B[:], l1b[:])
        L2G = res.tile([128, 128], F32); nc.sync.dma_start(L2G[:], l2g[:])
        L2B = res.tile([128, 128], F32); nc.sync.dma_start(L2B[:], l2b[:])
        IDR = res.tile([128, 128], F32R); nc.sync.dma_start(IDR[:], idr[:])
        IDF = res.tile([128, 128], F32); nc.sync.dma_start(IDF[:], idf[:])
        ZCOL = res.tile([128, 1], F32)
        nc.vector.memset(ZCOL[:], 0.0)
        ONEF = res.tile([128, 1], F32)
        nc.vector.memset(ONEF[:], 1.0)
        ONER = res.tile([128, 1], F32R)
        nc.vector.tensor_copy(ONER[:], ONEF[:])
        ONES16 = res.tile([1, 16], F32R)
        nc.vector.tensor_copy(ONES16[:], ONEF[0:1, :].broadcast_to([1, 16]))

        # per-b persistent activations
        XPOS = [res.tile([128, L], F32R, name=f"XPOS{b}") for b in range(BPC)]
        XSRC = [res.tile([128, 8, 128], F32, name=f"XSRC{b}") for b in range(BPC)]
        QT = [res.tile([128, L], F32R, name=f"QT{b}") for b in range(BPC)]
        KT = [res.tile([128, L], F32R, name=f"KT{b}") for b in range(BPC)]
        VH = [[res.tile([128, H, 17], F32R, name=f"VH{b}_{lt}") for lt in range(8)]
              for b in range(BPC)]
        ATB = [res.tile([128, L], F32, name=f"ATB{b}") for b in range(BPC)]

        # ================= PHASE A: convs =================
        with tc.tile_pool(name="pha", bufs=1) as pa, \
             tc.tile_pool(name="pha_ps", bufs=1, space="PSUM") as pap:
            for b in range(BPC):
                xsrcT = pa.tile([128, L], F32R, name=f"xsrcT{b}", tag="xsrcT", bufs=1)
                for lt in range(2):
                    pc2 = pap.tile([128, 512], F32, name="pc2", bufs=2)
                    for c4t in range(4):
                        for cig in range(4):
                            pc1 = pap.tile([128, 4, 512], F32, name="pc1", bufs=1)
                            gt = pa.tile([128, 4, 512], F32R, name="gt", bufs=2)
                            for j in range(4):
                                ci = cig * 4 + j
                                x7 = pa.tile([7, 512], F32R, name="x7", tag="x7",
                                             bufs=8)
                                src = bass.AP(xpad.tensor,
                                              (b * C_IN + ci) * (L + 6) + lt * 512,
                                              [[1, 7], [1, 512]])
                                nc.sync.dma_start(x7[:], src)
                                nc.tensor.matmul(pc1[:, j, :],
                                                 W1[:, c4t * 128:(c4t + 1) * 128],
                                                 x7[:], start=True, stop=True)
                            nc.scalar.activation(
                                gt[:].rearrange("p a b -> p (a b)"),
                                pc1[:].rearrange("p a b -> p (a b)"),
                                AF.Gelu, scale=S1[:, c4t:c4t + 1],
                                bias=B1[:, c4t:c4t + 1])
                            for j in range(4):
                                kt64 = c4t * 16 + cig * 4 + j
                                nc.tensor.matmul(pc2[:], W2L[:, kt64, :],
                                                 gt[:, j, :],
                                                 start=(kt64 == 0),
                                                 stop=(kt64 == 63))
                    nc.scalar.activation(xsrcT[:, lt * 512:(lt + 1) * 512],
                                         pc2[:], AF.Gelu,
                                         scale=S2[:, 0:1], bias=B2[:, 0:1])
                # x_pos^T = x_src^T + pe^T
                nc.vector.tensor_tensor(XPOS[b][:], xsrcT[:], PET[:], op=OP.add)
                # transpose x_src^T -> x_src [l, d]
                for lt in range(8):
                    pt = pap.tile([128, 128], F32R, name="ptA", tag="ptA", bufs=2)
                    nc.tensor.transpose(pt[:], xsrcT[:, lt * 128:(lt + 1) * 128],
                                        IDR[:])
                    nc.vector.tensor_copy(XSRC[b][:, lt, :], pt[:])

        # ================= PHASE B: projections =================
        if "B" in PH:
         with tc.tile_pool(name="phb", bufs=1) as pb, \
             tc.tile_pool(name="phb_ps", bufs=1, space="PSUM") as pbp:
            for b in range(BPC):
                for (wt, dst) in ((WQT, QT[b]), (WKT, KT[b])):
                    pp = pbp.tile([128, 512], F32, name="ppj", tag="ppj", bufs=4)
                    pp2 = pbp.tile([128, 512], F32, name="ppj2", tag="ppj", bufs=4)
                    nc.tensor.matmul(pp[:], wt[:], XPOS[b][:, 0:512],
                                     start=True, stop=True)
                    nc.tensor.matmul(pp2[:], wt[:], XPOS[b][:, 512:1024],
                                     start=True, stop=True)
                    nc.vector.tensor_copy(dst[:, 0:512], pp[:])
                    nc.vector.tensor_copy(dst[:, 512:1024], pp2[:])
                for lt in range(8):
                    pv = pbp.tile([128, 128], F32, name="pv", tag="pv", bufs=2)
                    nc.tensor.matmul(pv[:], XPOS[b][:, lt * 128:(lt + 1) * 128],
                                     WVT[:], start=True, stop=True)
                    vh = VH[b][lt]
                    nc.vector.tensor_copy(
                        vh[:].rearrange("p h d -> p (h d)")[:, 0:136].rearrange(
                            "p (h d) -> p h d", d=17)[:, :, 0:16],
                        pv[:].rearrange("p (h d) -> p h d", d=16))
                    nc.vector.tensor_copy(vh[:, :, 16:17],
                                          ONER[:, None, :].broadcast_to([128, 8, 1]))

        # ================= PHASE C: attention =================
        if "C" in PH:
         with tc.tile_pool(name="phc", bufs=1) as pc, \
             tc.tile_pool(name="phc_ps", bufs=1, space="PSUM") as pcp:
            for h in range(H):
                tb = pc.tile([128, 15, 128], F32R, name="tb", tag="tb", bufs=2)
                nc.sync.dma_start(
                    tb[:], bass.AP(ttr.tensor, h * 2047 + 127,
                                   [[H * 2047, 128], [128, 15], [1, 128]]))
                for b in range(BPC):
                    qth = pc.tile([16, L], F32R, name="qth", tag="qth", bufs=2)
                    kth = pc.tile([16, L], F32R, name="kth", tag="kth", bufs=2)
                    nc.sync.dma_start(qth[:], QT[b][16 * h:16 * h + 16, :])
                    nc.sync.dma_start(kth[:], KT[b][16 * h:16 * h + 16, :])
                    # BV = (B_h @ V)^T  [16, 1024]
                    pbv = pcp.tile([16, 1024], F32, name="pbv", tag="pbv", bufs=1)
                    for qh in range(2):
                        for kt in range(8):
                            d0 = 4 * qh - kt + 7
                            nc.tensor.matmul(
                                pbv[:, qh * 512:(qh + 1) * 512],
                                VH[b][kt][:, h, 0:16],
                                tb[:, d0:d0 + 4, :].rearrange("p a b -> p (a b)"),
                                start=(kt == 0), stop=(kt == 7))
                    bvs = pc.tile([16, 1024], F32, name="bvs", tag="bvs", bufs=2)
                    nc.vector.tensor_copy(bvs[:], pbv[:])
                    for qh in range(2):
                        pev = pcp.tile([17, 512], F32, name="pev", tag="pev",
                                       bufs=1)
                        for ktg in range(2):
                            pss = pcp.tile([128, 4, 512], F32, name="pss",
                                           tag="pss", bufs=1)
                            e4 = pc.tile([128, 4, 512], F32R, name="e4",
                                         tag="e4", bufs=2)
                            for j in range(4):
                                kt = ktg * 4 + j
                                nc.tensor.matmul(
                                    pss[:, j, :],
                                    kth[:, kt * 128:(kt + 1) * 128],
                                    qth[:, qh * 512:(qh + 1) * 512],
                                    start=True, stop=True)
                            nc.scalar.activation(
                                e4[:].rearrange("p a b -> p (a b)"),
                                pss[:].rearrange("p a b -> p (a b)"),
                                AF.Exp, scale=SCALE)
                            for j in range(4):
                                kt = ktg * 4 + j
                                nc.tensor.matmul(pev[:], VH[b][kt][:, h, :],
                                                 e4[:, j, :],
                                                 start=(kt == 0), stop=(kt == 7))
                        evs = pc.tile([17, 512], F32, name="evs", tag="evs",
                                      bufs=2)
                        nc.vector.tensor_copy(evs[:], pev[:])
                        rr = pc.tile([1, 512], F32R, name="rr", tag="rr", bufs=2)
                        rrf = pc.tile([1, 512], F32, name="rrf", tag="rrf", bufs=2)
                        nc.sync.dma_start(rrf[:], evs[16:17, :])
                        nc.vector.reciprocal(rrf[:], rrf[:])
                        nc.vector.tensor_copy(rr[:], rrf[:])
                        prb = pcp.tile([16, 512], F32, name="prb", tag="prb",
                                       bufs=1)
                        nc.tensor.matmul(prb[:], ONES16[:], rr[:],
                                         start=True, stop=True)
                        tmp = pc.tile([16, 512], F32, name="tmp", tag="tmp",
                                      bufs=2)
                        nc.vector.tensor_tensor(tmp[:], evs[0:16, :], prb[:],
                                                op=OP.mult)
                        nc.vector.tensor_tensor(tmp[:], tmp[:],
                                                bvs[:, qh * 512:(qh + 1) * 512],
                                                op=OP.add)
                        nc.sync.dma_start(
                            ATB[b][16 * h:16 * h + 16, qh * 512:(qh + 1) * 512],
                            tmp[:])

        # ================= PHASE D: merge + LN + FF =================
        if "D" in PH:
         with tc.tile_pool(name="phd", bufs=1) as pd, \
             tc.tile_pool(name="phd_ps", bufs=1, space="PSUM") as pdp:
            for b in range(BPC):
                at = pd.tile([128, 8, 128], F32, name=f"at{b}", tag="at", bufs=1)
                for lt in range(8):
                    pt = pdp.tile([128, 128], F32, name="ptD", tag="ptD", bufs=2)
                    nc.tensor.transpose(pt[:], ATB[b][:, lt * 128:(lt + 1) * 128],
                                        IDF[:])
                    nc.vector.tensor_copy(at[:, lt, :], pt[:])
                if DL < 2:
                    continue
                # LN_att
                aln = pd.tile([128, 8, 128], F32, name=f"aln{b}", tag="aln", bufs=1)
                _layer_norm_tiles(nc, pd, lambda lt: at[:, lt, :], LAG, LAB,
                                  lambda lt: aln[:, lt, :], f"la{b}")
                # residual + LN1 -> att
                r1 = pd.tile([128, 8, 128], F32, name=f"r1_{b}", tag="r1", bufs=1)
                for lt in range(8):
                    nc.vector.tensor_tensor(r1[:, lt, :], aln[:, lt, :],
                                            XSRC[b][:, lt, :], op=OP.add)
                att = pd.tile([128, 8, 128], F32, name=f"att{b}", tag="attb", bufs=1)
                _layer_norm_tiles(nc, pd, lambda lt: r1[:, lt, :], L1G, L1B,
                                  lambda lt: att[:, lt, :], f"l1{b}")
                if DL < 3:
                    continue
                # att^T via PE transpose
                attT = pd.tile([128, L], F32R, name=f"attT{b}", tag="attT", bufs=1)
                for lt in range(8):
                    pt = pdp.tile([128, 128], F32, name="ptD2", tag="ptD", bufs=2)
                    nc.tensor.transpose(pt[:], att[:, lt, :], IDF[:])
                    nc.vector.tensor_copy(attT[:, lt * 128:(lt + 1) * 128], pt[:])
                # ff1^T = relu(W1 @ att^T + b1)
                r1t = pd.tile([128, 4, L], F32R, name=f"r1t{b}", tag="r1t", bufs=1)
                for t in range(4):
                    pf = pdp.tile([128, 1024], F32, name="pf", tag="pf", bufs=2)
                    nc.tensor.matmul(pf[:, 0:512], W1F[:, t, :], attT[:, 0:512],
                                     start=True, stop=True)
                    nc.tensor.matmul(pf[:, 512:1024], W1F[:, t, :],
                                     attT[:, 512:1024], start=True, stop=True)
                    nc.vector.tensor_scalar(
                        out=r1t[:, t, :], in0=pf[:], scalar1=B1F[:, t:t + 1],
                        scalar2=0.0, op0=OP.add, op1=OP.max)
                if DL < 4:
                    continue
                # ff2 + residual + LN2 -> y
                z = pd.tile([128, 8, 128], F32, name=f"z{b}", tag="z", bufs=1)
                for lt in range(8):
                    pf2 = pdp.tile([128, 128], F32, name="pf2", tag="pf2", bufs=2)
                    for t in range(4):
                        nc.tensor.matmul(pf2[:],
                                         r1t[:, t, lt * 128:(lt + 1) * 128],
                                         W2F[:, t, :],
                                         start=(t == 0), stop=(t == 3))
                    nc.vector.tensor_tensor(z[:, lt, :], pf2[:], att[:, lt, :],
                                            op=OP.add)
                    nc.vector.tensor_tensor(z[:, lt, :], z[:, lt, :], B2BC[:],
                                            op=OP.add)
                yb = pd.tile([128, 8, 128], F32, name=f"yb{b}", tag="yb", bufs=1)
                _layer_norm_tiles(nc, pd, lambda lt: z[:, lt, :], L2G, L2B,
                                  lambda lt: yb[:, lt, :], f"l2{b}")
                # transpose y -> y^T [d, l] and store
                yt = pd.tile([128, L], F32, name=f"yt{b}", tag="yt", bufs=1)
                for lt in range(8):
                    pt = pdp.tile([128, 128], F32, name="ptD3", tag="ptD", bufs=2)
                    nc.tensor.transpose(pt[:], yb[:, lt, :], IDF[:])
                    nc.vector.tensor_copy(yt[:, lt * 128:(lt + 1) * 128], pt[:])
                nc.sync.dma_start(yout[b], yt[:])

    nc.compile()
    return nc


def _host_prep(inputs):
    f = np.float32
    x = np.asarray(inputs["x"], f)
    s1 = np.asarray(inputs["bn1_g"], f) / math.sqrt(1.0 + EPS)
    b1 = np.asarray(inputs["conv1_b"], f) * s1 + np.asarray(inputs["bn1_b"], f)
    s2 = np.asarray(inputs["bn2_g"], f) / math.sqrt(1.0 + EPS)
    b2 = np.asarray(inputs["conv2_b"], f) * s2 + np.asarray(inputs["bn2_b"], f)

    xpad = np.pad(x, ((0, 0), (0, 0), (3, 3))).astype(f)

    w1 = np.asarray(inputs["conv1_w"], f)[:, 0, 0, :]        # [512, 7]
    w1r = np.ascontiguousarray(w1.T)                          # [7, 512]
    s1t = np.ascontiguousarray(s1.reshape(4, 128).T)          # [128, 4]
    b1e = np.ascontiguousarray(b1.reshape(4, 128).T)

    w2 = np.asarray(inputs["conv2_w"], f)[:, :, :, 0]         # [d, c4, ci]
    w2l = np.ascontiguousarray(
        w2.reshape(128, 4, 128, 16).transpose(2, 1, 3, 0).reshape(128, 64, 128))
    s2t = s2.reshape(128, 1).astype(f)
    b2e = b2.reshape(128, 1).astype(f)

    pos = np.arange(L, dtype=f)[:, None]
    div = np.exp(np.arange(0, D, 2, dtype=f) * f(-math.log(10000.0) / D))
    ang = (pos * div * f(D / L)).astype(f)
    pe = np.stack([np.sin(ang), np.cos(ang)], axis=-1).reshape(L, D).astype(f)
    peT = np.ascontiguousarray(pe.T)

    wqT = np.ascontiguousarray(np.asarray(inputs["wq"], f).T)
    wkT = np.ascontiguousarray(np.asarray(inputs["wk"], f).T)
    wvT = np.ascontiguousarray(np.asarray(inputs["wv"], f).T)

    rel = np.asarray(inputs["rel_table"], f)                  # [2047, H]
    ttr = np.zeros((128, H, 2047), f)
    for kp in range(128):
        ttr[kp, :, kp:] = rel[:2047 - kp, :].T
    ttr = np.ascontiguousarray(ttr)

    ff_w1 = np.asarray(inputs["ff_w1"], f)                    # [512, 128]
    w1ft = np.ascontiguousarray(ff_w1.reshape(4, 128, 128).transpose(2, 0, 1))
    b1ft = np.ascontiguousarray(np.asarray(inputs["ff_b1"], f).reshape(4, 128).T)
    ff_w2 = np.asarray(inputs["ff_w2"], f)                    # [128, 512]
    w2ft = np.ascontiguousarray(ff_w2.reshape(128, 4, 128).transpose(2, 1, 0))
    b2bc = np.ascontiguousarray(
        np.broadcast_to(np.asarray(inputs["ff_b2"], f)[None, :], (128, 128)))

    def bc(v):
        return np.ascontiguousarray(
            np.broadcast_to(np.asarray(v, f)[None, :], (128, 128)))

    ident = np.eye(128, dtype=f)

    common = dict(
        w1r=w1r, s1t=s1t, b1e=b1e, w2l=w2l, s2t=s2t, b2e=b2e, peT=peT,
        wqT=wqT, wkT=wkT, wvT=wvT, ttr=ttr, w1ft=w1ft, b1ft=b1ft, w2ft=w2ft,
        b2bc=b2bc, lag=bc(inputs["ln_att_g"]), lab=bc(inputs["ln_att_b"]),
        l1g=bc(inputs["ln1_g"]), l1b=bc(inputs["ln1_b"]),
        l2g=bc(inputs["ln2_g"]), l2b=bc(inputs["ln2_b"]),
        idr=ident, idf=ident)
    in_maps = []
    for c in range(N_CORES):
        m = dict(common)
        m["xpad"] = np.ascontiguousarray(xpad[c * BPC:(c + 1) * BPC])
        in_maps.append(m)
    return in_maps


def kernel(**inputs) -> np.ndarray:
    if "nc" not in _CACHE:
        _CACHE["nc"] = _build()
    nc = _CACHE["nc"]
    in_maps = _host_prep(inputs)
    res = run_bass_kernel_spmd(nc, in_maps, list(range(N_CORES)))
    out = np.concatenate([r["yout"] for r in res.results], axis=0)
    return out.astype(np.float32)
